# revision 20
# baseline (speedup 1.0000x reference)
"""Trainium2 Bass kernel for nn_MultiHeadGraphAttention (v5).

Multi-head graph attention (GAT-style), one head per NeuronCore:
    h_prime = einsum('nf,hfo->hno', h, w)
    attn    = softmax(where(adj, leakyrelu(s_i + d_j), -inf), axis=-1)
    out     = attn @ h_prime + b

Softmax is invariant to any per-column scale, so with x = s_i + d_j,
v = exp(d), v2 = exp(0.2 d), g = exp(-0.8 s):

    p[i,j] = m * exp(leakyrelu(x)) = exp(s_i) * q[j,i]
    q[j,i] = m * max(v_j, g_i * v2_j)       (exp(s_i) cancels in softmax)

out = (Vaug^T @ q) normalized by the ones-column.  Per-element work per
[128, 4096] chunk of q^T (j on partitions):

  - "ts" route:  ONE fused DVE tensor_scalar  max(G * v2_j, v_j)
  - "act" route: q = exp(0.8*relu(-x) + d_j) via 2 ScalarE passes
  plus one DVE tensor_tensor mask multiply (pair-batched for half the
  pairs, single for the rest -- an A/B experiment).

Preamble: hT loaded as 8 independent half-tiles split across both HWDGE
rings so the s/d matmuls start after ~2MB instead of the full 4MB;
s broadcast across partitions with a PE outer-product (ones x s_row)
into PSUM quarters, from which ScalarE emits G = exp(-0.8 s) (bf16) and
S (f32) directly -- no DRAM bounce.  d transposed into per-partition
columns with PE transposes.  h_prime bias is folded into its matmul via
a ones row; mask pair-DMAs are prefetched before the sd phase.
"""
import sys

if "/opt/trn_rl_repo" not in sys.path:
    sys.path.insert(0, "/opt/trn_rl_repo")

from contextlib import ExitStack

import ml_dtypes
import numpy as np

import concourse.bass as bass
import concourse.bacc as bacc
import concourse.tile as tile
from concourse import mybir
from concourse.bass_utils import run_bass_kernel_spmd

F32 = mybir.dt.float32
BF16 = mybir.dt.bfloat16
AF = mybir.ActivationFunctionType
ALU = mybir.AluOpType

N = 4096
F_IN = 256
N_HEAD = 8
F_OUT = 64
NEG = 0.2
NCH = N // 128        # 32 j-chunks
NPR = NCH // 2        # 16 chunk-pairs
NSL = N // 512        # 8 512-slices
FC = F_IN // 128      # 2 f-chunks
HB = 2048             # half of N (hT half-tile width)
VW = F_OUT + 1        # 65: V columns + ones column

# const blob layout (bf16, [128, BLOB_W]):
#   [0:4) wa2_hi (fc-major)   [4:8) wa2_lo   [8:136) w   [136:200) b row
BLOB_W = 200

# Per-chunk route: "ts" (fused DVE tensor_scalar) or "act" (2x ScalarE).
_ACT = {2, 3, 8, 9, 14, 15, 20, 21, 26, 27, 30}
ROUTES = ["act" if i in _ACT else "ts" for i in range(NCH)]
# pairs with a single fused [128, 8192] mask TT (others: two [128, 4096])
PAIR_TT = set(range(NPR))


def build_program(routes=ROUTES):
    nc = bacc.Bacc("TRN2", target_bir_lowering=False, debug=False)
    hT_hi = nc.dram_tensor("hT_hi", [F_IN, N], BF16, kind="ExternalInput").ap()
    hT_lo = nc.dram_tensor("hT_lo", [F_IN, N], BF16, kind="ExternalInput").ap()
    blob = nc.dram_tensor("blob", [128, BLOB_W], BF16, kind="ExternalInput").ap()
    maskp = nc.dram_tensor("maskp", [NPR, 128, 2 * N], BF16,
                           kind="ExternalInput").ap()
    eye2 = nc.dram_tensor("eye2", [2, 2], F32, kind="ExternalInput").ap()
    outT = nc.dram_tensor("outT", [F_OUT, N], F32, kind="ExternalOutput").ap()
    sv_dram = nc.dram_tensor("sv_scratch", [N], F32).ap()
    r_dram = nc.dram_tensor("r_scratch", [N], F32).ap()

    with tile.TileContext(nc) as tc, ExitStack() as ctx:
        const_pool = ctx.enter_context(tc.tile_pool(name="const", bufs=1))
        mask_pool = ctx.enter_context(tc.tile_pool(name="maskpl", bufs=4))
        pre_ctx = ExitStack()
        pre_pool = pre_ctx.enter_context(tc.tile_pool(name="pre", bufs=1))

        # ---- warmup matmuls to release the PE HAM clock gate (no data dep)
        # (own short-lived PSUM pool so it doesn't widen the preamble pools)
        wu_t = pre_pool.tile([128, 512], BF16, tag="wu")
        nc.vector.memset(wu_t[:, :], 0.0)
        with tc.tile_pool(name="pswu", bufs=1, space="PSUM") as pswu_pool:
            ps_wu = pswu_pool.tile([128, 512], F32, tag="pswu")
            for _ in range(14):
                nc.tensor.matmul(ps_wu[:, :], wu_t[:, 0:128], wu_t[:, :],
                                 start=True, stop=True)
        psw_pool = pre_ctx.enter_context(tc.tile_pool(name="psw", bufs=2, space="PSUM"))
        psS_pool = pre_ctx.enter_context(tc.tile_pool(name="psS", bufs=2, space="PSUM"))
        psv_pool = pre_ctx.enter_context(tc.tile_pool(name="psv", bufs=2, space="PSUM"))

        # ---- input loads: hi halves on sync ring, blob + lo halves on scalar
        # 8 independent half-tiles so the first sd slices start early.
        hThi = [[pre_pool.tile([128, HB], BF16, tag=f"hThi{fc}{ha}",
                               name=f"hThi{fc}{ha}")
                 for ha in range(2)] for fc in range(FC)]
        hTlo = [[pre_pool.tile([128, HB], BF16, tag=f"hTlo{fc}{ha}",
                               name=f"hTlo{fc}{ha}")
                 for ha in range(2)] for fc in range(FC)]
        blob_sb = const_pool.tile([128, BLOB_W], BF16, tag="blob")
        nc.scalar.dma_start(blob_sb[:, :], blob[:, :])
        for ha in range(2):
            for fc in range(FC):
                nc.sync.dma_start(hThi[fc][ha][:, :],
                                  hT_hi[fc * 128:(fc + 1) * 128,
                                        ha * HB:(ha + 1) * HB])
            for fc in range(FC):
                nc.scalar.dma_start(hTlo[fc][ha][:, :],
                                    hT_lo[fc * 128:(fc + 1) * 128,
                                          ha * HB:(ha + 1) * HB])
        eye2_sb = const_pool.tile([2, 2], F32, tag="eye2")
        nc.scalar.dma_start(eye2_sb[:, :], eye2[:, :])
        wa2hi = blob_sb[:, 0:4]
        wa2lo = blob_sb[:, 4:8]
        w_sb = blob_sb[:, 8:136]
        brow_sb = blob_sb[0:1, 136:136 + F_OUT]
        ones_row = const_pool.tile([1, 128], BF16, tag="ones_row")
        nc.vector.memset(ones_row[:, :], 1.0)

        # ---- early mask prefetch (before sd work floods the rings)
        m_tiles = []
        for pc in range(4):
            m_t = mask_pool.tile([128, 2 * N], BF16, tag="mt")
            if pc % 2 == 0:
                nc.sync.dma_start(m_t[:, :], maskp[pc, :, :])
            else:
                nc.scalar.dma_start(m_t[:, :], maskp[pc, :, :])
            m_tiles.append(m_t)

        # ---- s/d rows (hi/lo compensated); s broadcast via PE outer product
        s_bf_row = pre_pool.tile([1, N], BF16, tag="sbfrow")
        sdT = pre_pool.tile([2, N], F32, tag="sdT")
        S_b = const_pool.tile([128, N], F32, tag="Sb")
        G_b = const_pool.tile([128, N], BF16, tag="Gb")
        ps_S = None
        for sl in range(NSL):
            s0 = sl * 512
            ha, off = sl // 4, (sl % 4) * 512
            ps_sd = psw_pool.tile([2, 512], F32, tag="pssd")
            ci = 0
            for fc in range(FC):
                for (wa, ht) in ((wa2hi, hThi[fc][ha]), (wa2hi, hTlo[fc][ha]),
                                 (wa2lo, hThi[fc][ha])):
                    nc.tensor.matmul(ps_sd[:, :], wa[:, fc * 2:(fc + 1) * 2],
                                     ht[:, off:off + 512],
                                     start=(ci == 0), stop=(ci == 5))
                    ci += 1
            nc.scalar.copy(sdT[0:2, s0:s0 + 512], ps_sd[:, :])
            nc.vector.tensor_copy(s_bf_row[0:1, s0:s0 + 512], sdT[0:1, s0:s0 + 512])
            ps_S = psS_pool.tile([128, 512], F32, tag="psS")
            nc.tensor.matmul(ps_S[:, :], ones_row[:, :],
                             s_bf_row[0:1, s0:s0 + 512], start=True, stop=True)
            nc.scalar.activation(G_b[:, s0:s0 + 512], ps_S[:, :],
                                 AF.Exp, scale=-0.8)
            nc.scalar.copy(S_b[:, s0:s0 + 512], ps_S[:, :])

        # ---- d columns via PE transposes -> tables
        ps_dall = psw_pool.tile([128, 2 * NCH], F32, tag="psdall")
        for jc in range(NCH):
            nc.tensor.transpose(ps_dall[:, 2 * jc:2 * jc + 2],
                                sdT[0:2, jc * 128:(jc + 1) * 128], eye2_sb[:, :])
        dT_sb = const_pool.tile([128, NCH], F32, tag="dT")
        nc.vector.tensor_copy(dT_sb[:, :], ps_dall[:, 1::2])
        negd08 = const_pool.tile([128, NCH], F32, tag="negd08")
        nc.vector.tensor_scalar(negd08[:, :], dT_sb[:, :], -0.8, None, op0=ALU.mult)
        v_sb = const_pool.tile([128, NCH], F32, tag="v")
        nc.scalar.activation(v_sb[:, :], dT_sb[:, :], AF.Exp)
        v2_sb = const_pool.tile([128, NCH], F32, tag="v2")
        nc.scalar.activation(v2_sb[:, :], dT_sb[:, :], AF.Exp, scale=NEG)

        # ---- h_prime (V, bf16), bias folded in via ones row; copies on DVE
        V_sb = const_pool.tile([128, NCH * VW], BF16, tag="V")
        nc.vector.memset(V_sb[:, :], 1.0)
        for jc in range(NCH):
            ha, off = jc // 16, (jc % 16) * 128
            ps_v = psv_pool.tile([128, F_OUT], F32, tag="psv")
            for fc in range(FC):
                nc.tensor.matmul(ps_v[:, :], hThi[fc][ha][:, off:off + 128],
                                 w_sb[:, fc * F_OUT:(fc + 1) * F_OUT],
                                 start=(fc == 0), stop=False)
            nc.tensor.matmul(ps_v[:, :], ones_row[:, :], brow_sb[:, :],
                             start=False, stop=True)
            nc.vector.tensor_copy(V_sb[:, jc * VW: jc * VW + F_OUT], ps_v[:, :])

        # ---------------- attention j-loop over chunk PAIRS ----------------
        pre_ctx.close()
        loop_ctx = ExitStack()
        q_pool = loop_ctx.enter_context(tc.tile_pool(name="qp", bufs=3))
        r32_pool = loop_ctx.enter_context(tc.tile_pool(name="r32p", bufs=2))
        p_pool = loop_ctx.enter_context(tc.tile_pool(name="pp", bufs=2))
        psbig_pool = ctx.enter_context(tc.tile_pool(name="psbig", bufs=1, space="PSUM"))
        ps_A = psbig_pool.tile([VW, N], F32, tag="psA")
        for pc in range(NPR):
            if pc < 4:
                m_t = m_tiles[pc]
            else:
                m_t = mask_pool.tile([128, 2 * N], BF16, tag="mt")
                if pc % 2 == 0:
                    nc.sync.dma_start(m_t[:, :], maskp[pc, :, :])
                else:
                    nc.scalar.dma_start(m_t[:, :], maskp[pc, :, :])
            q_t = q_pool.tile([128, 2 * N], BF16, tag="qt")
            for half in (0, 1):
                jc = 2 * pc + half
                qs = slice(half * N, (half + 1) * N)
                if routes[jc] == "ts":
                    nc.vector.tensor_scalar(q_t[:, qs], G_b[:, :],
                                            v2_sb[:, jc:jc + 1], v_sb[:, jc:jc + 1],
                                            op0=ALU.mult, op1=ALU.max)
                else:
                    r_t = r32_pool.tile([128, N], F32, tag="rt")
                    nc.scalar.activation(r_t[:, :], S_b[:, :], AF.Relu,
                                         scale=-0.8, bias=negd08[:, jc:jc + 1])
                    nc.scalar.activation(q_t[:, qs], r_t[:, :], AF.Exp,
                                         bias=dT_sb[:, jc:jc + 1])
            p_t = p_pool.tile([128, 2 * N], BF16, tag="pt")
            if pc in PAIR_TT:
                nc.vector.tensor_tensor(p_t[:, :], q_t[:, :], m_t[:, :],
                                        op=ALU.mult)
            else:
                for half in (0, 1):
                    qs = slice(half * N, (half + 1) * N)
                    nc.vector.tensor_tensor(p_t[:, qs], q_t[:, qs], m_t[:, qs],
                                            op=ALU.mult)
            for half in (0, 1):
                jc = 2 * pc + half
                for k in range(NSL):
                    nc.tensor.matmul(
                        ps_A[:, k * 512:(k + 1) * 512],
                        V_sb[:, jc * VW:(jc + 1) * VW],
                        p_t[:, half * N + k * 512: half * N + (k + 1) * 512],
                        start=(jc == 0), stop=(jc == NCH - 1))

        # ---------------- tail: normalize (half-pipelined) ----------------
        loop_ctx.close()
        H2 = N // 2
        tail_pool = ctx.enter_context(tc.tile_pool(name="tail", bufs=1))
        Srow = tail_pool.tile([1, N], F32, tag="Srow")
        sres = tail_pool.tile([128, N // 128], F32, tag="sres")
        rres = tail_pool.tile([128, N // 128], F32, tag="rres")
        R_sb = tail_pool.tile([F_OUT, N], F32, tag="Rsb")
        F_sb = tail_pool.tile([F_OUT, N], F32, tag="Fsb")
        Q4 = N // 128 // 2  # 16 sres columns per half
        for half in range(2):
            hs = slice(half * H2, (half + 1) * H2)
            if half == 0:
                nc.scalar.copy(Srow[0:1, hs], ps_A[F_OUT:VW, hs])
            else:
                nc.vector.tensor_copy(Srow[0:1, hs], ps_A[F_OUT:VW, hs])
            nc.scalar.dma_start(sv_dram[hs], Srow[0:1, hs])
            nc.scalar.dma_start(
                sres[:, half * Q4:(half + 1) * Q4],
                sv_dram[hs].rearrange("(p q) -> p q", p=128))
            nc.vector.reciprocal(rres[:, half * Q4:(half + 1) * Q4],
                                 sres[:, half * Q4:(half + 1) * Q4])
            nc.scalar.dma_start(
                r_dram[hs].rearrange("(p q) -> p q", p=128),
                rres[:, half * Q4:(half + 1) * Q4])
            nc.sync.dma_start(R_sb[:, hs],
                              r_dram[None, hs].broadcast_to((F_OUT, H2)))
            nc.vector.tensor_tensor(F_sb[:, hs], ps_A[0:F_OUT, hs], R_sb[:, hs],
                                    op=ALU.mult)
            nc.scalar.dma_start(outT[:, hs], F_sb[:, hs])
    nc.compile()
    return nc


_CACHED_NC = None


def _get_nc():
    global _CACHED_NC
    if _CACHED_NC is None:
        _CACHED_NC = build_program()
    return _CACHED_NC


def _split_hilo(x):
    hi = x.astype(ml_dtypes.bfloat16)
    lo = (x - hi.astype(np.float32)).astype(ml_dtypes.bfloat16)
    return hi, lo


def _prep_inputs(h, adj, w, a_src, a_dst, b):
    h = np.asarray(h, dtype=np.float32)
    adj = np.asarray(adj)
    w = np.asarray(w, dtype=np.float32)
    a_src = np.asarray(a_src, dtype=np.float32)
    a_dst = np.asarray(a_dst, dtype=np.float32)
    b = np.asarray(b, dtype=np.float32)

    h_T = np.ascontiguousarray(h.T)
    hT_hi, hT_lo = _split_hilo(h_T)
    # pair-layout mask: maskp[pc, p, half*N + i] = adj[i, (2*pc+half)*128 + p]
    mT = np.ascontiguousarray(adj.T).astype(ml_dtypes.bfloat16)
    maskp = np.ascontiguousarray(
        mT.reshape(NPR, 2, 128, N).transpose(0, 2, 1, 3).reshape(NPR, 128, 2 * N))

    in_maps = []
    for c in range(N_HEAD):
        wa_src = (w[c] @ a_src[c])[:, 0]              # [F_IN]
        wa_dst = (w[c] @ a_dst[c])[:, 0]
        cols = np.stack([wa_src, wa_dst], axis=1)     # [F_IN, 2]
        wa_hi, wa_lo = _split_hilo(cols)
        blob = np.zeros((128, BLOB_W), dtype=ml_dtypes.bfloat16)
        blob[:, 0:2] = wa_hi[0:128]
        blob[:, 2:4] = wa_hi[128:256]
        blob[:, 4:6] = wa_lo[0:128]
        blob[:, 6:8] = wa_lo[128:256]
        blob[:, 8:72] = w[c][0:128].astype(ml_dtypes.bfloat16)
        blob[:, 72:136] = w[c][128:256].astype(ml_dtypes.bfloat16)
        blob[0, 136:136 + F_OUT] = b.astype(ml_dtypes.bfloat16)
        in_maps.append({
            "hT_hi": hT_hi,
            "hT_lo": hT_lo,
            "blob": blob,
            "maskp": maskp,
            "eye2": np.eye(2, dtype=np.float32),
        })
    return in_maps


def _run(in_maps, trace=False, **kwargs):
    nc = _get_nc()
    return run_bass_kernel_spmd(nc, in_maps, list(range(N_HEAD)), trace=trace, **kwargs)


def kernel(h, adj, w, a_src, a_dst, b):
    in_maps = _prep_inputs(h, adj, w, a_src, a_dst, b)
    res = _run(in_maps)
    out = np.stack([np.ascontiguousarray(res.results[c]["outT"].T)
                    for c in range(N_HEAD)])
    return out.astype(np.float32)
